# revision 38
# baseline (speedup 1.0000x reference)
"""MixerDiffAttention Trainium2 kernel (v2 — pipelined phase 2).

Sharding: 8 cores = 8 head-pairs (tensor parallel over head-pair dim).
Each core processes BOTH batches for its head-pair: the per-core weight
slice (768 qkv cols + 256 gate cols) stays SBUF-resident, and each core
produces the disjoint output slice y[:, :, hp*256:(hp+1)*256].

Per core, per batch:
  Phase 1 (per 128-token tile): qkv+gate matmul (fp32r, K=2048
    accumulated); q/k RMSNorm stats via ACT Square+accum; rstd via DVE
    Quake-seed Newton; RoPE on DVE; PE-transpose rope'd q/k to
    feature-major [hd, T]; v -> SBUF with an appended ones column (gives
    softmax row-sums for free); gate raw -> SBUF.
  Phase 2 (flat software-pipelined job stream): score matmuls for
    group g+1 are emitted BEFORE the AV matmuls of group g, so the PE
    never sits behind ACT's exp. Exps are batched 2 pairs per ACT
    instruction. The causal-diagonal mask is applied multiplicatively
    AFTER exp on the (otherwise idle) GPSIMD engine (keeps ACT inside
    the exp_and_friends table set -> no table swaps). SiLU gates are
    computed just-in-time one chunk ahead via sigma(g)=1/(1+exp(-g))
    (ACT Exp + DVE recip-approx + GPSIMD mult). The epilogue fuses
    diff-attention combine, gating, and group RMSNorm (sum-of-squares
    via one tensor_tensor_reduce).
"""
import sys
sys.path.insert(0, "/opt/trn_rl_repo")
import numpy as np
import concourse.bass as bass
from concourse import bacc
import concourse.tile as tile
from concourse import mybir
from concourse.bass_utils import run_bass_kernel_spmd

F32 = mybir.dt.float32
F32R = mybir.dt.float32r
BF16 = mybir.dt.bfloat16
AF = mybir.ActivationFunctionType
ALU = mybir.AluOpType

B, T, D, HD = 2, 2048, 2048, 128
KT = D // 128          # 16 contraction tiles
TT = T // 128          # 16 token tiles
CH = 256               # query-chunk width in phase 2
NCH = T // CH          # 8 chunks
N_CORES = 8
LAMBDA_INIT = 0.8 - 0.6 * float(np.exp(-0.3 * 6))
ONE_MINUS_LI = 1.0 - LAMBDA_INIT
SCALE = float(HD ** -0.5)
EPS = 1e-6


def _bcast_mid(ap, n):
    # [P, F] AP -> [P, n, F] with a zero-stride middle dim
    return bass.AP(tensor=ap.tensor, offset=ap.offset,
                   ap=[ap.ap[0], [0, n], *ap.ap[1:]])


def _rsqrt_dve(nc, pool, ss_ap, width, mean_div, tag, iters=2):
    """rstd = (ss/mean_div + EPS) ** -0.5 entirely on DVE.

    Quake-III bit-trick seed + Newton iterations (2 it: ~5e-6 rel err;
    1 it: ~1.7e-3 max rel err); avoids ACT Ln/Sqrt so the whole kernel
    stays inside one ACT table set."""
    I32 = mybir.dt.int32
    ms = pool.tile([128, width], F32, name=tag + "_ms")
    nc.vector.tensor_scalar(out=ms[:], in0=ss_ap, scalar1=1.0 / mean_div,
                            scalar2=EPS, op0=ALU.mult, op1=ALU.add)
    iv = pool.tile([128, width], I32, name=tag + "_iv")
    nc.vector.tensor_scalar(out=iv[:], in0=ms[:].bitcast(I32), scalar1=1,
                            scalar2=None, op0=ALU.logical_shift_right)
    y = pool.tile([128, width], F32, name=tag + "_y")
    nc.vector.tensor_scalar(out=y[:].bitcast(I32), in0=iv[:], scalar1=-1,
                            scalar2=0x5F3759DF, op0=ALU.mult, op1=ALU.add)
    a = pool.tile([128, width], F32, name=tag + "_a")
    u = pool.tile([128, width], F32, name=tag + "_u")
    for _ in range(iters):
        nc.vector.tensor_mul(a[:], y[:], y[:])
        nc.vector.tensor_mul(a[:], a[:], ms[:])
        nc.vector.tensor_scalar(out=u[:], in0=a[:], scalar1=-0.5, scalar2=1.5,
                                op0=ALU.mult, op1=ALU.add)
        nc.vector.tensor_mul(y[:], y[:], u[:])
    return y


def build(tt=TT, nb=B, phases=2):
    nch = tt * 128 // CH
    nc = bacc.Bacc("TRN2", target_bir_lowering=False, debug=False,
                   num_devices=N_CORES)
    xt_d = nc.dram_tensor("xt", [nb, D, tt * 128], BF16, kind="ExternalInput").ap()
    w_d = nc.dram_tensor("wcat", [D, 1024], BF16, kind="ExternalInput").ap()
    cos_d = nc.dram_tensor("cos", [tt * 128, 64], F32, kind="ExternalInput").ap()
    sin_d = nc.dram_tensor("sin", [tt * 128, 64], F32, kind="ExternalInput").ap()
    mask_d = nc.dram_tensor("masks", [128, 2, CH], F32R, kind="ExternalInput").ap()
    id_d = nc.dram_tensor("ident", [128, 128], BF16, kind="ExternalInput").ap()
    ones_d = nc.dram_tensor("ones", [128, 4], F32R, kind="ExternalInput").ap()
    y_d = nc.dram_tensor("y", [nb, tt * 128, 256], F32, kind="ExternalOutput").ap()

    with tile.TileContext(nc) as tc:
        with tc.tile_pool(name="bigs", bufs=1) as bigs, \
             tc.tile_pool(name="consts", bufs=1) as consts, \
             tc.tile_pool(name="xtp", bufs=4) as xtp:
            # ---- weights first: the k=0..1 slices gate the first matmul ----
            wcat = bigs.tile([128, KT, 1024], BF16)
            w_v = w_d.rearrange("(k p) c -> p k c", p=128)

            # ---- 256-token "super tile" loads (bf16, 512B runs) ----
            xT_pre = {}

            def load_super(b, s):
                xv = xt_d[b].rearrange("(k p) t -> p k t", p=128)
                xp = xtp.tile([128, KT, 256], BF16, name="xT_s")
                for kh in range(2):
                    nc.sync.dma_start(
                        xp[:, kh * 8:(kh + 1) * 8, :],
                        xv[:, kh * 8:(kh + 1) * 8, s * 256:(s + 1) * 256])
                xT_pre[(b, s)] = xp

            # first x half + first weight slices gate the first matmuls;
            # interleave so the PE starts within ~4us
            xv0 = xt_d[0].rearrange("(k p) t -> p k t", p=128)
            xp0 = xtp.tile([128, KT, 256], BF16, name="xT_s")
            nc.sync.dma_start(xp0[:, 0:8, :], xv0[:, 0:8, 0:256])
            for k in range(2):
                nc.sync.dma_start(wcat[:, k, :], w_v[:, k, :])
            nc.sync.dma_start(xp0[:, 8:16, :], xv0[:, 8:16, 0:256])
            xT_pre[(0, 0)] = xp0
            for k in range(2, KT):
                nc.sync.dma_start(wcat[:, k, :], w_v[:, k, :])
            # ---- small constants (needed only after the first projection) ----
            cos_sb = consts.tile([128, tt, 64], F32)
            nc.sync.dma_start(cos_sb[:], cos_d.rearrange("(t p) f -> p t f", p=128))
            sin_sb = consts.tile([128, tt, 64], F32)
            nc.sync.dma_start(sin_sb[:], sin_d.rearrange("(t p) f -> p t f", p=128))
            mask_sb = consts.tile([128, 2, CH], F32R)
            nc.sync.dma_start(mask_sb[:], mask_d)
            id_sb = consts.tile([128, 128], BF16)
            nc.sync.dma_start(id_sb[:], id_d)
            ones_sb = consts.tile([128, 4], F32R)
            nc.sync.dma_start(ones_sb[:], ones_d)
            load_super(0, 1)
            load_super(0, 2)
            load_super(0, 3)

            # ---- per-batch persistent (reused sequentially) ----
            qkT = bigs.tile([128, tt, 4, 128], BF16)    # t-major; rows q1,q2,k1,k2
            v_sb = bigs.tile([128, tt, 260], F32R)      # [tok, v(256)|1|0 pad]
            g_sb = bigs.tile([128, tt, 256], F32)       # gate (raw -> silu'd JIT)

            for b in range(nb):
                # ================= Phase 1 =================
                with tc.tile_pool(name="p1s", bufs=2) as p1s, \
                     tc.tile_pool(name="p1t", bufs=3) as p1t, \
                     tc.tile_pool(name="tp_ps", bufs=2, space="PSUM") as tp_ps, \
                     tc.tile_pool(name="mm_ps", bufs=3, space="PSUM") as mm_ps:
                    # ones column for every tile in one strided write
                    nc.vector.tensor_copy(v_sb[:, :, 256:260],
                                          _bcast_mid(ones_sb[:], tt))

                    def p1_transp(t, qrot):
                        # q/k transposes -> one PSUM bank -> qkT (emitted one
                        # tile late so the PE never waits on the rope chain)
                        tq = tp_ps.tile([128, 512], BF16, name="tq")
                        for h in range(4):
                            nc.tensor.transpose(tq[:, h * 128:(h + 1) * 128],
                                                qrot[:, h, :], id_sb[:])
                        nc.vector.tensor_copy(qkT[:, t, :, :],
                                              tq[:].rearrange("p (h d) -> p h d", h=4))

                    pending_tp = None
                    for t in range(tt):
                        s, half = t // 2, t % 2
                        if half == 0 and (b, s) not in xT_pre:
                            load_super(b, s)
                        xT_t = xT_pre[(b, s)]
                        if half == 1:
                            del xT_pre[(b, s)]
                            # prefetch 3 supers ahead
                            if s + 3 < tt // 2 and (b, s + 3) not in xT_pre:
                                load_super(b, s + 3)
                        xsl = slice(half * 128, half * 128 + 128)
                        qk_ps = mm_ps.tile([128, 512], F32, name="qk_ps")
                        vg_ps = mm_ps.tile([128, 512], F32, name="vg_ps")
                        for k in range(KT):
                            nc.tensor.matmul(qk_ps[:], xT_t[:, k, xsl], wcat[:, k, 0:512],
                                             start=(k == 0), stop=(k == KT - 1))
                        for k in range(KT):
                            nc.tensor.matmul(vg_ps[:], xT_t[:, k, xsl], wcat[:, k, 512:1024],
                                             start=(k == 0), stop=(k == KT - 1))
                        if pending_tp is not None:
                            p1_transp(*pending_tp)
                        # ---- q/k rmsnorm stats FIRST on ACT (they gate the
                        # rsqrt -> qrot -> transpose chain) ----
                        ss = p1t.tile([128, 4], F32, name="ss")
                        sq_scr = p1t.tile([128, 128], F32, name="sq_scr")
                        for h in range(4):
                            nc.scalar.activation(sq_scr[:], qk_ps[:, h * 128:(h + 1) * 128],
                                                 AF.Square, accum_out=ss[:, h:h + 1])
                        # ---- rope in f32 straight from PSUM (single bf16
                        # rounding happens at qrot) ----
                        qk_v = qk_ps[:].rearrange("p (h d) -> p h d", h=4)
                        h1, h2 = qk_v[:, :, 0:64], qk_v[:, :, 64:128]
                        cos_b = _bcast_mid(cos_sb[:, t, :], 4)
                        sin_b = _bcast_mid(sin_sb[:, t, :], 4)
                        ra = p1t.tile([128, 4, 64], F32, name="ra")
                        rb = p1t.tile([128, 4, 64], F32, name="rb")
                        rot = p1t.tile([128, 4, 128], F32, name="rot")
                        nc.vector.tensor_mul(ra[:], h1, cos_b)
                        nc.vector.tensor_mul(rb[:], h2, sin_b)
                        nc.vector.tensor_add(rot[:, :, 0:64], ra[:], rb[:])
                        nc.vector.tensor_mul(ra[:], h2, cos_b)
                        nc.vector.tensor_mul(rb[:], h1, sin_b)
                        nc.vector.tensor_sub(rot[:, :, 64:128], ra[:], rb[:])
                        rstd = _rsqrt_dve(nc, p1t, ss[:], 4, HD, "rq", iters=2)
                        qrot = p1t.tile([128, 4, 128], BF16, name="qrot")
                        for h in range(4):
                            nc.vector.tensor_scalar_mul(qrot[:, h, :], in0=rot[:, h, :],
                                                        scalar1=rstd[:, h:h + 1])
                        pending_tp = (t, qrot)
                        # ---- v / raw gate copies + SiLU gate: emitted LAST
                        # (consumed only in phase 2, keeps the ACT queue
                        # clear of the critical chain) ----
                        nc.scalar.copy(v_sb[:, t, 0:256], vg_ps[:, 0:256])
                        nc.scalar.copy(g_sb[:, t, :], vg_ps[:, 256:512])
                        ge = p1t.tile([128, 256], F32, name="ge")
                        nc.scalar.activation(ge[:], g_sb[:, t, :], AF.Exp,
                                             scale=-1.0)
                        gd = p1t.tile([128, 256], F32, name="gd")
                        nc.vector.tensor_scalar(out=gd[:], in0=ge[:], scalar1=1.0,
                                                scalar2=None, op0=ALU.add)
                        gr = p1t.tile([128, 256], F32, name="gr")
                        nc.vector.reciprocal_approx_fast(out=gr[:], in_=gd[:])
                        nc.vector.tensor_mul(g_sb[:, t, :], g_sb[:, t, :], gr[:])
                    p1_transp(*pending_tp)
                # prefetch next batch's first supers during phase 2
                if b + 1 < nb:
                    for s in range(2):
                        load_super(b + 1, s)
                if phases < 2:
                    with tc.tile_pool(name="dump", bufs=2) as dump:
                        for t in range(tt):
                            d_t = dump.tile([128, 256], F32, name="d_t")
                            nc.vector.tensor_copy(d_t[:], v_sb[:, t, 0:256])
                            nc.vector.tensor_add(d_t[:], d_t[:], g_sb[:, t, :])
                            nc.sync.dma_start(
                                y_d[b, t * 128:(t + 1) * 128, :], d_t[:])
                    continue
                # ================= Phase 2 =================
                with tc.tile_pool(name="p2s", bufs=3) as p2s, \
                     tc.tile_pool(name="p2e", bufs=3) as p2e, \
                     tc.tile_pool(name="sc_ps", bufs=2, space="PSUM") as sc_ps, \
                     tc.tile_pool(name="av_ps", bufs=4, space="PSUM") as av_ps:
                    # --- job list: groups of <=2 score pairs; diagonal is its
                    # own group (needs the causal mask) -------------------
                    groups = []
                    for c in range(nch):
                        for var in range(2):
                            offd = list(range(c))
                            for i in range(0, len(offd), 2):
                                groups.append((c, var, offd[i:i + 2], False))
                            groups.append((c, var, [c], True))

                    sc_tiles = {}

                    def emit_sc(gi):
                        c, var, prs, diag = groups[gi]
                        scp = sc_ps.tile([128, 4, CH], F32, name="sc")
                        qch = qkT[:, 2 * c:2 * c + 2, var, :]
                        for pi, jp in enumerate(prs):
                            for jj in range(2):
                                nc.tensor.matmul(
                                    scp[:, 2 * pi + jj, :],
                                    qkT[:, 2 * jp + jj, 2 + var, :],
                                    qch, start=True, stop=True)
                        sc_tiles[gi] = scp

                    emit_sc(0)
                    yps = {}
                    for gi, (c, var, prs, diag) in enumerate(groups):
                        if var == 0 and prs[0] == 0:
                            for v2 in range(2):
                                for m in range(2):
                                    yps[(v2, m)] = av_ps.tile([128, 260], F32,
                                                              name="yacc")
                        n = 2 * len(prs)
                        scp = sc_tiles.pop(gi)
                        probs = p2s.tile([128, 4, CH], F32R, name="probs")
                        nc.scalar.activation(probs[:, 0:n, :], scp[:, 0:n, :],
                                             AF.Exp, scale=SCALE)
                        if diag:
                            nc.vector.tensor_mul(probs[:, 0:2, :],
                                                 probs[:, 0:2, :], mask_sb[:])
                        # emit next group's scores ahead of this group's AV
                        if gi + 1 < len(groups):
                            emit_sc(gi + 1)
                        for pi, jp in enumerate(prs):
                            for jj in range(2):
                                j = 2 * jp + jj
                                for m in range(2):
                                    nc.tensor.matmul(
                                        yps[(var, m)][:],
                                        probs[:, 2 * pi + jj, m * 128:(m + 1) * 128],
                                        v_sb[:, j, :],
                                        start=(j == 0), stop=(j == 2 * c + 1))
                        if not (diag and var == 1):
                            continue
                        # ---- epilogue for chunk c ----
                        ssy = p2e.tile([128, 2], F32, name="ssy")
                        ygs = []
                        for m in range(2):
                            y1p, y2p = yps[(0, m)], yps[(1, m)]
                            # v col 256 = 1 -> s1; col 257 = -1/lam -> r2n is
                            # one reciprocal away
                            r1 = p2e.tile([128, 1], F32, name="r1")
                            r2n = p2e.tile([128, 1], F32, name="r2n")
                            nc.vector.reciprocal(r1[:], y1p[:, 256:257])
                            nc.vector.reciprocal(r2n[:], y2p[:, 257:258])
                            t1 = p2e.tile([128, 256], F32, name="t1")
                            nc.vector.tensor_scalar_mul(t1[:], in0=y1p[:, 0:256],
                                                        scalar1=r1[:])
                            yt = p2e.tile([128, 256], F32, name="yt")
                            nc.vector.scalar_tensor_tensor(
                                yt[:], y2p[:, 0:256], r2n[:], t1[:],
                                op0=ALU.mult, op1=ALU.add)
                            yg = p2e.tile([128, 256], F32, name="yg", bufs=2)
                            nc.vector.tensor_mul(yg[:], yt[:],
                                                 g_sb[:, 2 * c + m, :])
                            if c == nch - 1:
                                # tail chunk: ACT is idle; keep DVE chain short
                                sq = p2e.tile([128, 256], F32, name="sq2")
                                nc.scalar.activation(sq[:], yg[:], AF.Square,
                                                     accum_out=ssy[:, m:m + 1])
                            else:
                                sq = p2e.tile([128, 256], F32, name="sq2")
                                nc.vector.tensor_mul(sq[:], yg[:], yg[:])
                                nc.vector.tensor_reduce(
                                    ssy[:, m:m + 1], sq[:],
                                    axis=mybir.AxisListType.X, op=ALU.add)
                            ygs.append(yg)
                        rsy = _rsqrt_dve(nc, p2e, ssy[:], 2, 256, "ry", iters=2)
                        for m in range(2):
                            qt = 2 * c + m
                            out_t = p2e.tile([128, 256], F32, name="out_t")
                            nc.vector.tensor_scalar(
                                out=out_t[:], in0=ygs[m][:],
                                scalar1=rsy[:, m:m + 1], scalar2=ONE_MINUS_LI,
                                op0=ALU.mult, op1=ALU.mult)
                            nc.sync.dma_start(
                                y_d[b, qt * 128:(qt + 1) * 128, :], out_t[:])
    nc.compile()
    return nc


_NC = None


def prep_in_maps(hidden_states, W_qkv, lambda_q1, lambda_k1, lambda_q2,
                 lambda_k2, W_g):
    import ml_dtypes
    bf16 = ml_dtypes.bfloat16
    x = np.asarray(hidden_states, dtype=np.float32)
    xt = np.ascontiguousarray(x.transpose(0, 2, 1)).astype(bf16)
    W_qkv = np.asarray(W_qkv, dtype=np.float32)
    W_g = np.asarray(W_g, dtype=np.float32)

    t_ar = np.arange(T, dtype=np.float32)
    inv_freq = (1.0 / 10000.0 ** (np.arange(0, HD, 2, dtype=np.float32) / HD)
                ).astype(np.float32)
    freqs = np.outer(t_ar, inv_freq).astype(np.float32)
    cos = np.cos(freqs).astype(np.float32)
    sin = np.sin(freqs).astype(np.float32)

    # multiplicative 0/1 causal mask (applied to probs AFTER exp)
    masks = np.empty((128, 2, CH), dtype=np.float32)
    kk = np.arange(128)[:, None]
    qq = np.arange(CH)[None, :]
    for m in range(2):
        masks[:, m, :] = np.where(m * 128 + kk <= qq, 1.0, 0.0)
    
    ident = np.eye(128, dtype=bf16)

    lam1 = np.exp(np.sum(np.asarray(lambda_q1, np.float32)
                         * np.asarray(lambda_k1, np.float32), axis=-1))
    lam2 = np.exp(np.sum(np.asarray(lambda_q2, np.float32)
                         * np.asarray(lambda_k2, np.float32), axis=-1))
    lam = (lam1 - lam2 + LAMBDA_INIT).astype(np.float32)   # [8]

    in_maps = []
    for c in range(N_CORES):
        base = 2 * c * 384
        w_cols = [
            W_qkv[:, base:base + 128],            # q1
            W_qkv[:, base + 384:base + 512],      # q2
            W_qkv[:, base + 128:base + 256],      # k1
            W_qkv[:, base + 512:base + 640],      # k2
            W_qkv[:, base + 256:base + 384],      # v1
            W_qkv[:, base + 640:base + 768],      # v2
            W_g[:, c * 256:(c + 1) * 256],        # gate
        ]
        wcat = np.ascontiguousarray(np.concatenate(w_cols, axis=1)).astype(bf16)
        ones = np.zeros((128, 4), dtype=np.float32)
        ones[:, 0] = 1.0
        ones[:, 1] = -1.0 / lam[c]
        in_maps.append({
            "xt": xt, "wcat": wcat, "cos": cos, "sin": sin,
            "masks": masks, "ident": ident, "ones": ones,
        })

    return in_maps


def kernel(hidden_states, W_qkv, lambda_q1, lambda_k1, lambda_q2, lambda_k2,
           W_g, **run_kwargs):
    global _NC
    if _NC is None:
        _NC = build()
    in_maps = prep_in_maps(hidden_states, W_qkv, lambda_q1, lambda_k1,
                           lambda_q2, lambda_k2, W_g)
    res = run_bass_kernel_spmd(_NC, in_maps, core_ids=list(range(N_CORES)),
                               **run_kwargs)
    out = np.empty((B, T, D), dtype=np.float32)
    for c in range(N_CORES):
        out[:, :, c * 256:(c + 1) * 256] = res.results[c]["y"]
    if run_kwargs:
        return out, res
    return out


# revision 64
# speedup vs baseline: 1.0906x; 1.0906x over previous
"""MixerDiffAttention Trainium2 kernel (v3 — deep-pipelined, bf16 I/O).

Sharding: 8 cores = 8 head-pairs (tensor parallel over head-pair dim).
Each core processes BOTH batches for its head-pair: the per-core weight
slice (768 qkv cols + 256 gate cols) stays SBUF-resident, and each core
produces the disjoint output slice y[:, :, hp*256:(hp+1)*256].

Key scheduling facts (from the timeline cost model): matmul cost =
out_free_size x dtype_rate (contraction depth is free), engines execute
in per-engine program order, and reopened tile pools carry WAR deps on
the previous scope's readers -- so the P2 SBUF pools are hoisted to the
outer scope. q/k transposes run entirely on the DMA xbar
(dma_start_transpose, bf16): no PE time, no PSUM bank, no copyback.

Per core, per batch:
  Phase 1 (per 128-token tile; x and W stream in as bf16, 256-token
    512B-run DMAs): qk projection matmuls first, then v|gate (the
    qk-stats chain starts half a tile early); qk staged to SBUF f32
    (frees the PSUM bank); RMSNorm stats via ACT Square+accum; rstd via
    DVE Quake-seed Newton (2 it); RoPE on DVE in f32; single bf16
    rounding at the rstd-scale; feature-major q/k via DMA-xbar
    transposes; v (+ones column
    for softmax row sums) and raw gate copied by ACT one tile late;
    SiLU gate via sigma=1/(1+exp(-g)): ACT Exp, Pool add, DVE
    recip-approx, Pool mult (all off the critical chain).
  Phase 2 (flat software-pipelined group stream): score matmuls for
    group g+1 are emitted BEFORE the AV matmuls of group g, so the PE
    never sits behind ACT's exp. Exps are batched 2 score-pairs per ACT
    instruction (exp_and_friends table set only -> no table swaps); the
    causal-diagonal slice is masked multiplicatively after exp (exact
    0/1 f32 on DVE); the diagonal AV block that is fully causal-masked
    is skipped outright. The epilogue overlaps attention: y1's normalize
    starts when var0's accumulators finish; the diff combine, SiLU
    gating, and group RMSNorm (rsqrt absorbs the 1-lambda_init factor)
    finish after var1, with sum-of-squares on DVE (ACT on the tail
    chunk where ACT is idle).
"""
import sys
sys.path.insert(0, "/opt/trn_rl_repo")
import numpy as np
import concourse.bass as bass
from concourse import bacc
import concourse.tile as tile
from concourse import mybir
from concourse.bass_utils import run_bass_kernel_spmd

F32 = mybir.dt.float32
F32R = mybir.dt.float32r
BF16 = mybir.dt.bfloat16
AF = mybir.ActivationFunctionType
ALU = mybir.AluOpType

B, T, D, HD = 2, 2048, 2048, 128
KT = D // 128          # 16 contraction tiles
TT = T // 128          # 16 token tiles
CH = 256               # query-chunk width in phase 2
NCH = T // CH          # 8 chunks
N_CORES = 8
LAMBDA_INIT = 0.8 - 0.6 * float(np.exp(-0.3 * 6))
ONE_MINUS_LI = 1.0 - LAMBDA_INIT
SCALE = float(HD ** -0.5)
EPS = 1e-6


def _bcast_mid(ap, n):
    # [P, F] AP -> [P, n, F] with a zero-stride middle dim
    return bass.AP(tensor=ap.tensor, offset=ap.offset,
                   ap=[ap.ap[0], [0, n], *ap.ap[1:]])


def _rsqrt_dve(nc, pool, ss_ap, width, mean_div, tag, iters=2, eps=EPS):
    """rstd = (ss/mean_div + EPS) ** -0.5 entirely on DVE.

    Quake-III bit-trick seed + Newton iterations (2 it: ~5e-6 rel err;
    1 it: ~1.7e-3 max rel err); avoids ACT Ln/Sqrt so the whole kernel
    stays inside one ACT table set."""
    I32 = mybir.dt.int32
    ms = pool.tile([128, width], F32, name=tag + "_ms")
    nc.vector.tensor_scalar(out=ms[:], in0=ss_ap, scalar1=1.0 / mean_div,
                            scalar2=eps, op0=ALU.mult, op1=ALU.add)
    iv = pool.tile([128, width], I32, name=tag + "_iv")
    nc.vector.tensor_scalar(out=iv[:], in0=ms[:].bitcast(I32), scalar1=1,
                            scalar2=None, op0=ALU.logical_shift_right)
    y = pool.tile([128, width], F32, name=tag + "_y")
    nc.vector.tensor_scalar(out=y[:].bitcast(I32), in0=iv[:], scalar1=-1,
                            scalar2=0x5F3759DF, op0=ALU.mult, op1=ALU.add)
    a = pool.tile([128, width], F32, name=tag + "_a")
    u = pool.tile([128, width], F32, name=tag + "_u")
    for _ in range(iters):
        nc.vector.tensor_mul(a[:], y[:], y[:])
        nc.vector.tensor_mul(a[:], a[:], ms[:])
        nc.vector.tensor_scalar(out=u[:], in0=a[:], scalar1=-0.5, scalar2=1.5,
                                op0=ALU.mult, op1=ALU.add)
        nc.vector.tensor_mul(y[:], y[:], u[:])
    return y


def build(tt=TT, nb=B, phases=2):
    nch = tt * 128 // CH
    nc = bacc.Bacc("TRN2", target_bir_lowering=False, debug=False,
                   num_devices=N_CORES)
    xt_d = nc.dram_tensor("xt", [nb, D, tt * 128], BF16, kind="ExternalInput").ap()
    w_d = nc.dram_tensor("wcat", [D, 1024], BF16, kind="ExternalInput").ap()
    cos_d = nc.dram_tensor("cos", [tt * 128, 64], F32, kind="ExternalInput").ap()
    sin_d = nc.dram_tensor("sin", [tt * 128, 64], F32, kind="ExternalInput").ap()
    mask_d = nc.dram_tensor("masks", [128, 2, CH], F32R, kind="ExternalInput").ap()
    id_d = nc.dram_tensor("ident", [128, 128], BF16, kind="ExternalInput").ap()
    ones_d = nc.dram_tensor("ones", [128, 4], F32R, kind="ExternalInput").ap()
    y_d = nc.dram_tensor("y", [nb, tt * 128, 256], F32, kind="ExternalOutput").ap()

    with tile.TileContext(nc) as tc:
        with tc.tile_pool(name="bigs", bufs=1) as bigs, \
             tc.tile_pool(name="consts", bufs=1) as consts, \
             tc.tile_pool(name="p2s", bufs=3) as p2s, \
             tc.tile_pool(name="p2e", bufs=3) as p2e, \
             tc.tile_pool(name="xtp", bufs=4) as xtp:
            # ---- weights first: the k=0..1 slices gate the first matmul ----
            wcat = bigs.tile([128, KT, 1024], BF16)
            w_v = w_d.rearrange("(k p) c -> p k c", p=128)

            # ---- 256-token "super tile" loads (bf16, 512B runs) ----
            xT_pre = {}

            def load_super(b, s):
                xv = xt_d[b].rearrange("(k p) t -> p k t", p=128)
                xp = xtp.tile([128, KT, 256], BF16, name="xT_s")
                for kh in range(2):
                    nc.sync.dma_start(
                        xp[:, kh * 8:(kh + 1) * 8, :],
                        xv[:, kh * 8:(kh + 1) * 8, s * 256:(s + 1) * 256])
                xT_pre[(b, s)] = xp

            # first x half + first weight slices gate the first matmuls;
            # interleave so the PE starts within ~4us
            xv0 = xt_d[0].rearrange("(k p) t -> p k t", p=128)
            xp0 = xtp.tile([128, KT, 256], BF16, name="xT_s")
            nc.sync.dma_start(xp0[:, 0:4, :], xv0[:, 0:4, 0:256])
            nc.sync.dma_start(wcat[:, 0, :], w_v[:, 0, :])
            nc.sync.dma_start(xp0[:, 4:8, :], xv0[:, 4:8, 0:256])
            nc.sync.dma_start(wcat[:, 1, :], w_v[:, 1, :])
            nc.sync.dma_start(xp0[:, 8:16, :], xv0[:, 8:16, 0:256])
            xT_pre[(0, 0)] = xp0
            for k in range(2, KT):
                nc.sync.dma_start(wcat[:, k, :], w_v[:, k, :])
            load_super(0, 1)
            # ---- small constants (needed only after the first projection) ----
            cos_sb = consts.tile([128, tt, 64], F32)
            nc.sync.dma_start(cos_sb[:], cos_d.rearrange("(t p) f -> p t f", p=128))
            sin_sb = consts.tile([128, tt, 64], F32)
            nc.sync.dma_start(sin_sb[:], sin_d.rearrange("(t p) f -> p t f", p=128))
            id_sb = consts.tile([128, 128], BF16)
            nc.sync.dma_start(id_sb[:], id_d)
            load_super(0, 2)
            mask_sb = consts.tile([128, 2, CH], F32R)
            nc.sync.dma_start(mask_sb[:], mask_d)
            ones_sb = consts.tile([128, 4], F32R)
            nc.sync.dma_start(ones_sb[:], ones_d)
            load_super(0, 3)

            # ---- per-batch persistent (reused sequentially) ----
            qkT = bigs.tile([128, tt, 4, 128], BF16)    # t-major; rows q1,q2,k1,k2
            v_sb = bigs.tile([128, tt, 260], F32R)      # [tok, v(256)|1|0 pad]
            g_sb = bigs.tile([128, tt, 256], F32)       # gate (raw -> silu'd JIT)

            for b in range(nb):
                # ================= Phase 1 =================
                with tc.tile_pool(name="p1t", bufs=3) as p1t, \
                     tc.tile_pool(name="mm_ps", bufs=3, space="PSUM") as mm_ps:
                    # ones column for every tile in one strided write
                    nc.vector.tensor_copy(v_sb[:, :, 256:260],
                                          _bcast_mid(ones_sb[:], tt))

                    def p1_transp(t, qrot):
                        # q/k transposes entirely on the DMA xbar: no PE
                        # time, no PSUM bank, no copyback
                        for h in range(4):
                            nc.sync.dma_start_transpose(qkT[:, t, h, :],
                                                        qrot[:, h, :])

                    pending_vg = None

                    def p1_vg(t, vg_ps):
                        # v / raw gate copies + SiLU gate; deferred one tile
                        # so the next tile's squares lead the ACT queue
                        nc.scalar.copy(v_sb[:, t, 0:256], vg_ps[:, 0:256])
                        nc.scalar.copy(g_sb[:, t, :], vg_ps[:, 256:512])
                        ge = p1t.tile([128, 256], F32, name="ge")
                        nc.scalar.activation(ge[:], g_sb[:, t, :], AF.Exp,
                                             scale=-1.0)
                        gd = p1t.tile([128, 256], F32, name="gd")
                        nc.gpsimd.tensor_scalar(out=gd[:], in0=ge[:], scalar1=1.0,
                                                scalar2=None, op0=ALU.add)
                        gr = p1t.tile([128, 256], F32, name="gr")
                        nc.vector.reciprocal_approx_fast(out=gr[:], in_=gd[:])
                        nc.gpsimd.tensor_mul(g_sb[:, t, :], g_sb[:, t, :], gr[:])

                    for t in range(tt):
                        s, half = t // 2, t % 2
                        if half == 0 and (b, s) not in xT_pre:
                            load_super(b, s)
                        xT_t = xT_pre[(b, s)]
                        if half == 1:
                            del xT_pre[(b, s)]
                            # prefetch 3 supers ahead
                            if s + 3 < tt // 2 and (b, s + 3) not in xT_pre:
                                load_super(b, s + 3)
                        xsl = slice(half * 128, half * 128 + 128)
                        qk_ps = mm_ps.tile([128, 512], F32, name="qk_ps")
                        vg_ps = mm_ps.tile([128, 512], F32, name="vg_ps")
                        for k in range(KT):
                            nc.tensor.matmul(qk_ps[:], xT_t[:, k, xsl], wcat[:, k, 0:512],
                                             start=(k == 0), stop=(k == KT - 1))
                        for k in range(KT):
                            nc.tensor.matmul(vg_ps[:], xT_t[:, k, xsl], wcat[:, k, 512:1024],
                                             start=(k == 0), stop=(k == KT - 1))
                        # ---- q/k rmsnorm stats FIRST on ACT (they gate the
                        # rsqrt -> qrot -> transpose chain) ----
                        ss = p1t.tile([128, 4], F32, name="ss")
                        sq_scr = p1t.tile([128, 128], F32, name="sq_scr")
                        for h in range(4):
                            nc.scalar.activation(sq_scr[:], qk_ps[:, h * 128:(h + 1) * 128],
                                                 AF.Square, accum_out=ss[:, h:h + 1])
                        # ---- stage qk to SBUF in f32 (frees the PSUM bank
                        # early; single bf16 rounding happens at qrot) ----
                        qksb = p1t.tile([128, 4, 128], F32, name="qksb")
                        nc.scalar.copy(qksb[:],
                                       qk_ps[:].rearrange("p (h d) -> p h d", h=4))
                        h1, h2 = qksb[:, :, 0:64], qksb[:, :, 64:128]
                        cos_b = _bcast_mid(cos_sb[:, t, :], 4)
                        sin_b = _bcast_mid(sin_sb[:, t, :], 4)
                        ra = p1t.tile([128, 4, 64], F32, name="ra")
                        rb = p1t.tile([128, 4, 64], F32, name="rb")
                        rot = p1t.tile([128, 4, 128], F32, name="rot")
                        nc.vector.tensor_mul(ra[:], h1, cos_b)
                        nc.vector.tensor_mul(rb[:], h2, sin_b)
                        nc.vector.tensor_add(rot[:, :, 0:64], ra[:], rb[:])
                        nc.vector.tensor_mul(ra[:], h2, cos_b)
                        nc.vector.tensor_mul(rb[:], h1, sin_b)
                        nc.vector.tensor_sub(rot[:, :, 64:128], ra[:], rb[:])
                        rstd = _rsqrt_dve(nc, p1t, ss[:], 4, HD, "rq", iters=2)
                        qrot = p1t.tile([128, 4, 128], BF16, name="qrot")
                        for h in range(4):
                            nc.vector.tensor_scalar_mul(qrot[:, h, :], in0=rot[:, h, :],
                                                        scalar1=rstd[:, h:h + 1])
                        p1_transp(t, qrot)
                        if pending_vg is not None:
                            p1_vg(*pending_vg)
                        pending_vg = (t, vg_ps)
                    p1_vg(*pending_vg)
                # prefetch next batch's first supers during phase 2
                if b + 1 < nb:
                    for s in range(2):
                        load_super(b + 1, s)
                if phases < 2:
                    with tc.tile_pool(name="dump", bufs=2) as dump:
                        for t in range(tt):
                            d_t = dump.tile([128, 256], F32, name="d_t")
                            nc.vector.tensor_copy(d_t[:], v_sb[:, t, 0:256])
                            nc.vector.tensor_add(d_t[:], d_t[:], g_sb[:, t, :])
                            nc.sync.dma_start(
                                y_d[b, t * 128:(t + 1) * 128, :], d_t[:])
                    continue
                # ================= Phase 2 =================
                with tc.tile_pool(name="sc_ps", bufs=2, space="PSUM") as sc_ps, \
                     tc.tile_pool(name="av_ps", bufs=4, space="PSUM") as av_ps:
                    # --- job list: groups of <=2 score pairs; diagonal is its
                    # own group (needs the causal mask) -------------------
                    groups = []
                    for c in range(nch):
                        for var in range(2):
                            prs_all = list(range(c + 1))
                            for i in range(0, len(prs_all), 2):
                                grp = prs_all[i:i + 2]
                                groups.append((c, var, grp, c in grp))

                    sc_tiles = {}

                    def emit_sc(gi):
                        c, var, prs, diag = groups[gi]
                        scp = sc_ps.tile([128, 4, CH], F32, name="sc")
                        qch = qkT[:, 2 * c:2 * c + 2, var, :]
                        for pi, jp in enumerate(prs):
                            for jj in range(2):
                                nc.tensor.matmul(
                                    scp[:, 2 * pi + jj, :],
                                    qkT[:, 2 * jp + jj, 2 + var, :],
                                    qch, start=True, stop=True)
                        sc_tiles[gi] = scp

                    emit_sc(0)
                    yps = {}
                    for gi, (c, var, prs, diag) in enumerate(groups):
                        if var == 0 and prs[0] == 0:
                            for v2 in range(2):
                                for m in range(2):
                                    yps[(v2, m)] = av_ps.tile([128, 260], F32,
                                                              name="yacc")
                        n = 2 * len(prs)
                        scp = sc_tiles.pop(gi)
                        probs = p2s.tile([128, 4, CH], F32R, name="probs")
                        nc.scalar.activation(probs[:, 0:n, :], scp[:, 0:n, :],
                                             AF.Exp, scale=SCALE)
                        if diag:
                            pi = prs.index(c)
                            nc.vector.tensor_mul(
                                probs[:, 2 * pi:2 * pi + 2, :],
                                probs[:, 2 * pi:2 * pi + 2, :], mask_sb[:])
                        # emit next group's scores ahead of this group's AV
                        if gi + 1 < len(groups):
                            emit_sc(gi + 1)
                        for pi, jp in enumerate(prs):
                            for jj in range(2):
                                j = 2 * jp + jj
                                for m in range(2):
                                    if j == 2 * c + 1 and m == 0:
                                        # fully-masked diagonal block: probs
                                        # are exactly zero there -> skip
                                        continue
                                    nc.tensor.matmul(
                                        yps[(var, m)][:],
                                        probs[:, 2 * pi + jj, m * 128:(m + 1) * 128],
                                        v_sb[:, j, :],
                                        start=(j == 0),
                                        stop=(j == 2 * c + 1 - (1 - m)))
                        if diag and var == 0:
                            # var0 accumulators are complete: start the
                            # normalize of y1 while var1's attention runs
                            pre_ep = {}
                            for m in range(2):
                                y1p = yps[(0, m)]
                                r1 = p2e.tile([128, 1], F32, name="r1")
                                nc.vector.reciprocal(r1[:], y1p[:, 256:257])
                                t1 = p2e.tile([128, 256], F32, name="t1")
                                nc.vector.tensor_scalar_mul(
                                    t1[:], in0=y1p[:, 0:256], scalar1=r1[:])
                                pre_ep[m] = t1
                        if not (diag and var == 1):
                            continue
                        # ---- epilogue for chunk c ----
                        ssy = p2e.tile([128, 2], F32, name="ssy")
                        ygs = []
                        for m in range(2):
                            y2p = yps[(1, m)]
                            # v col 257 = -1/lam -> r2n is one recip away
                            r2n = p2e.tile([128, 1], F32, name="r2n")
                            nc.vector.reciprocal(r2n[:], y2p[:, 257:258])
                            t1 = pre_ep[m]
                            yt = p2e.tile([128, 256], F32, name="yt")
                            nc.vector.scalar_tensor_tensor(
                                yt[:], y2p[:, 0:256], r2n[:], t1[:],
                                op0=ALU.mult, op1=ALU.add)
                            yg = p2e.tile([128, 256], F32, name="yg", bufs=2)
                            nc.vector.tensor_mul(yg[:], yt[:],
                                                 g_sb[:, 2 * c + m, :])
                            if c == nch - 1:
                                # tail chunk: ACT is idle; keep DVE chain short
                                sq = p2e.tile([128, 256], F32, name="sq2")
                                nc.scalar.activation(sq[:], yg[:], AF.Square,
                                                     accum_out=ssy[:, m:m + 1])
                            else:
                                sq = p2e.tile([128, 256], F32, name="sq2")
                                nc.vector.tensor_mul(sq[:], yg[:], yg[:])
                                nc.vector.tensor_reduce(
                                    ssy[:, m:m + 1], sq[:],
                                    axis=mybir.AxisListType.X, op=ALU.add)
                            ygs.append(yg)
                        # rsy absorbs the (1-lambda_init) factor:
                        # (ms/C^2)^-0.5 = C * ms^-0.5
                        CI2 = 1.0 / (ONE_MINUS_LI * ONE_MINUS_LI)
                        rsy = _rsqrt_dve(nc, p2e, ssy[:], 2, 256 / CI2, "ry",
                                         iters=2, eps=EPS * CI2)
                        for m in range(2):
                            qt = 2 * c + m
                            out_t = p2e.tile([128, 256], F32, name="out_t")
                            nc.vector.tensor_scalar_mul(
                                out_t[:], in0=ygs[m][:],
                                scalar1=rsy[:, m:m + 1])
                            nc.sync.dma_start(
                                y_d[b, qt * 128:(qt + 1) * 128, :], out_t[:])
    nc.compile()
    return nc


_NC = None


def prep_in_maps(hidden_states, W_qkv, lambda_q1, lambda_k1, lambda_q2,
                 lambda_k2, W_g):
    import ml_dtypes
    bf16 = ml_dtypes.bfloat16
    x = np.asarray(hidden_states, dtype=np.float32)
    xt = np.ascontiguousarray(x.transpose(0, 2, 1)).astype(bf16)
    W_qkv = np.asarray(W_qkv, dtype=np.float32)
    W_g = np.asarray(W_g, dtype=np.float32)

    t_ar = np.arange(T, dtype=np.float32)
    inv_freq = (1.0 / 10000.0 ** (np.arange(0, HD, 2, dtype=np.float32) / HD)
                ).astype(np.float32)
    freqs = np.outer(t_ar, inv_freq).astype(np.float32)
    cos = np.cos(freqs).astype(np.float32)
    sin = np.sin(freqs).astype(np.float32)

    # multiplicative 0/1 causal mask (applied to probs AFTER exp)
    masks = np.empty((128, 2, CH), dtype=np.float32)
    kk = np.arange(128)[:, None]
    qq = np.arange(CH)[None, :]
    for m in range(2):
        masks[:, m, :] = np.where(m * 128 + kk <= qq, 1.0, 0.0)
    
    ident = np.eye(128, dtype=bf16)

    lam1 = np.exp(np.sum(np.asarray(lambda_q1, np.float32)
                         * np.asarray(lambda_k1, np.float32), axis=-1))
    lam2 = np.exp(np.sum(np.asarray(lambda_q2, np.float32)
                         * np.asarray(lambda_k2, np.float32), axis=-1))
    lam = (lam1 - lam2 + LAMBDA_INIT).astype(np.float32)   # [8]

    in_maps = []
    for c in range(N_CORES):
        base = 2 * c * 384
        w_cols = [
            W_qkv[:, base:base + 128],            # q1
            W_qkv[:, base + 384:base + 512],      # q2
            W_qkv[:, base + 128:base + 256],      # k1
            W_qkv[:, base + 512:base + 640],      # k2
            W_qkv[:, base + 256:base + 384],      # v1
            W_qkv[:, base + 640:base + 768],      # v2
            W_g[:, c * 256:(c + 1) * 256],        # gate
        ]
        wcat = np.ascontiguousarray(np.concatenate(w_cols, axis=1)).astype(bf16)
        ones = np.zeros((128, 4), dtype=np.float32)
        ones[:, 0] = 1.0
        ones[:, 1] = -1.0 / lam[c]
        in_maps.append({
            "xt": xt, "wcat": wcat, "cos": cos, "sin": sin,
            "masks": masks, "ident": ident, "ones": ones,
        })

    return in_maps


def kernel(hidden_states, W_qkv, lambda_q1, lambda_k1, lambda_q2, lambda_k2,
           W_g, **run_kwargs):
    global _NC
    if _NC is None:
        _NC = build()
    in_maps = prep_in_maps(hidden_states, W_qkv, lambda_q1, lambda_k1,
                           lambda_q2, lambda_k2, W_g)
    res = run_bass_kernel_spmd(_NC, in_maps, core_ids=list(range(N_CORES)),
                               **run_kwargs)
    out = np.empty((B, T, D), dtype=np.float32)
    for c in range(N_CORES):
        out[:, :, c * 256:(c + 1) * 256] = res.results[c]["y"]
    if run_kwargs:
        return out, res
    return out


# revision 77
# speedup vs baseline: 1.1156x; 1.0230x over previous
"""MixerDiffAttention Trainium2 kernel (v3 — deep-pipelined, bf16 I/O).

Sharding: 8 cores = 8 head-pairs (tensor parallel over head-pair dim).
Each core processes BOTH batches for its head-pair: the per-core weight
slice (768 qkv cols + 256 gate cols) stays SBUF-resident, and each core
produces the disjoint output slice y[:, :, hp*256:(hp+1)*256].

Key scheduling facts (from the timeline cost model): matmul cost =
out_free_size x dtype_rate (contraction depth is free), engines execute
in per-engine program order, and reopened tile pools carry WAR deps on
the previous scope's readers -- so the P2 SBUF pools are hoisted to the
outer scope. q/k transposes run entirely on the DMA xbar
(dma_start_transpose, bf16): no PE time, no PSUM bank, no copyback.

Per core, per batch:
  Phase 1 (per 128-token tile; x and W stream in as bf16, 256-token
    512B-run DMAs): qk projection matmuls first, then v|gate (the
    qk-stats chain starts half a tile early); qk staged to SBUF f32
    (frees the PSUM bank); RMSNorm stats via ACT Square+accum; rstd via
    DVE Quake-seed Newton (2 it); RoPE on DVE in f32; single bf16
    rounding at the rstd-scale; feature-major q/k via DMA-xbar
    transposes; v (+ones column
    for softmax row sums) and raw gate copied by ACT one tile late;
    SiLU gate via sigma=1/(1+exp(-g)): ACT Exp, Pool add, DVE
    recip-approx, Pool mult (all off the critical chain).
  Phase 2 (flat software-pipelined group stream, var0/var1 streams
    interleaved per chunk): score matmuls for group g+1 are emitted
    BEFORE the AV matmuls of group g, so the PE never sits behind
    ACT's exp. Exps are batched 2 score-pairs per ACT
    instruction (exp_and_friends table set only -> no table swaps); the
    causal-diagonal slice is masked multiplicatively after exp (exact
    0/1 f32 on DVE); the diagonal AV block that is fully causal-masked
    is skipped outright. The epilogue overlaps attention: y1's normalize
    starts when var0's accumulators finish; the diff combine, SiLU
    gating, and group RMSNorm (rsqrt absorbs the 1-lambda_init factor)
    finish after var1, with sum-of-squares on DVE (ACT on the tail
    chunk where ACT is idle).
"""
import sys
sys.path.insert(0, "/opt/trn_rl_repo")
import numpy as np
import concourse.bass as bass
from concourse import bacc
import concourse.tile as tile
from concourse import mybir
from concourse.bass_utils import run_bass_kernel_spmd

F32 = mybir.dt.float32
F32R = mybir.dt.float32r
BF16 = mybir.dt.bfloat16
AF = mybir.ActivationFunctionType
ALU = mybir.AluOpType

B, T, D, HD = 2, 2048, 2048, 128
KT = D // 128          # 16 contraction tiles
TT = T // 128          # 16 token tiles
CH = 256               # query-chunk width in phase 2
NCH = T // CH          # 8 chunks
N_CORES = 8
LAMBDA_INIT = 0.8 - 0.6 * float(np.exp(-0.3 * 6))
ONE_MINUS_LI = 1.0 - LAMBDA_INIT
SCALE = float(HD ** -0.5)
EPS = 1e-6


def _bcast_mid(ap, n):
    # [P, F] AP -> [P, n, F] with a zero-stride middle dim
    return bass.AP(tensor=ap.tensor, offset=ap.offset,
                   ap=[ap.ap[0], [0, n], *ap.ap[1:]])


def _rsqrt_dve(nc, pool, ss_ap, width, mean_div, tag, iters=2, eps=EPS):
    """rstd = (ss/mean_div + EPS) ** -0.5 entirely on DVE.

    Quake-III bit-trick seed + Newton iterations (2 it: ~5e-6 rel err;
    1 it: ~1.7e-3 max rel err); avoids ACT Ln/Sqrt so the whole kernel
    stays inside one ACT table set."""
    I32 = mybir.dt.int32
    ms = pool.tile([128, width], F32, name=tag + "_ms")
    nc.vector.tensor_scalar(out=ms[:], in0=ss_ap, scalar1=1.0 / mean_div,
                            scalar2=eps, op0=ALU.mult, op1=ALU.add)
    iv = pool.tile([128, width], I32, name=tag + "_iv")
    nc.vector.tensor_scalar(out=iv[:], in0=ms[:].bitcast(I32), scalar1=1,
                            scalar2=None, op0=ALU.logical_shift_right)
    y = pool.tile([128, width], F32, name=tag + "_y")
    nc.vector.tensor_scalar(out=y[:].bitcast(I32), in0=iv[:], scalar1=-1,
                            scalar2=0x5F3759DF, op0=ALU.mult, op1=ALU.add)
    a = pool.tile([128, width], F32, name=tag + "_a")
    u = pool.tile([128, width], F32, name=tag + "_u")
    for _ in range(iters):
        nc.vector.tensor_mul(a[:], y[:], y[:])
        nc.vector.tensor_mul(a[:], a[:], ms[:])
        nc.vector.tensor_scalar(out=u[:], in0=a[:], scalar1=-0.5, scalar2=1.5,
                                op0=ALU.mult, op1=ALU.add)
        nc.vector.tensor_mul(y[:], y[:], u[:])
    return y


def build(tt=TT, nb=B, phases=2):
    nch = tt * 128 // CH
    nc = bacc.Bacc("TRN2", target_bir_lowering=False, debug=False,
                   num_devices=N_CORES)
    xt_d = nc.dram_tensor("xt", [nb, D, tt * 128], BF16, kind="ExternalInput").ap()
    w_d = nc.dram_tensor("wcat", [D, 1024], BF16, kind="ExternalInput").ap()
    cos_d = nc.dram_tensor("cos", [tt * 128, 64], F32, kind="ExternalInput").ap()
    sin_d = nc.dram_tensor("sin", [tt * 128, 64], F32, kind="ExternalInput").ap()
    mask_d = nc.dram_tensor("masks", [128, 2, CH], F32R, kind="ExternalInput").ap()
    id_d = nc.dram_tensor("ident", [128, 128], BF16, kind="ExternalInput").ap()
    ones_d = nc.dram_tensor("ones", [128, 4], F32R, kind="ExternalInput").ap()
    y_d = nc.dram_tensor("y", [nb, tt * 128, 256], F32, kind="ExternalOutput").ap()

    with tile.TileContext(nc) as tc:
        with tc.tile_pool(name="bigs", bufs=1) as bigs, \
             tc.tile_pool(name="consts", bufs=1) as consts, \
             tc.tile_pool(name="p2s", bufs=5) as p2s, \
             tc.tile_pool(name="p2e", bufs=5) as p2e, \
             tc.tile_pool(name="xtp", bufs=5) as xtp:
            # ---- weights first: the k=0..1 slices gate the first matmul ----
            wcat = bigs.tile([128, KT, 1024], BF16)
            w_v = w_d.rearrange("(k p) c -> p k c", p=128)

            # ---- 256-token "super tile" loads (bf16, 512B runs) ----
            xT_pre = {}

            def load_super(b, s):
                xv = xt_d[b].rearrange("(k p) t -> p k t", p=128)
                xp = xtp.tile([128, KT, 256], BF16, name="xT_s")
                for kh in range(2):
                    nc.sync.dma_start(
                        xp[:, kh * 8:(kh + 1) * 8, :],
                        xv[:, kh * 8:(kh + 1) * 8, s * 256:(s + 1) * 256])
                xT_pre[(b, s)] = xp

            # first x half + first weight slices gate the first matmuls;
            # interleave so the PE starts within ~4us
            xv0 = xt_d[0].rearrange("(k p) t -> p k t", p=128)
            xp0 = xtp.tile([128, KT, 256], BF16, name="xT_s")
            nc.sync.dma_start(xp0[:, 0:4, :], xv0[:, 0:4, 0:256])
            nc.sync.dma_start(wcat[:, 0, :], w_v[:, 0, :])
            nc.sync.dma_start(xp0[:, 4:8, :], xv0[:, 4:8, 0:256])
            nc.sync.dma_start(wcat[:, 1, :], w_v[:, 1, :])
            nc.sync.dma_start(xp0[:, 8:16, :], xv0[:, 8:16, 0:256])
            xT_pre[(0, 0)] = xp0
            for k in range(2, KT):
                nc.sync.dma_start(wcat[:, k, :], w_v[:, k, :])
            load_super(0, 1)
            # ---- small constants (needed only after the first projection) ----
            cos_sb = consts.tile([128, tt, 64], F32)
            nc.sync.dma_start(cos_sb[:], cos_d.rearrange("(t p) f -> p t f", p=128))
            sin_sb = consts.tile([128, tt, 64], F32)
            nc.sync.dma_start(sin_sb[:], sin_d.rearrange("(t p) f -> p t f", p=128))
            id_sb = consts.tile([128, 128], BF16)
            nc.sync.dma_start(id_sb[:], id_d)
            load_super(0, 2)
            mask_sb = consts.tile([128, 2, CH], F32R)
            nc.sync.dma_start(mask_sb[:], mask_d)
            ones_sb = consts.tile([128, 4], F32R)
            nc.sync.dma_start(ones_sb[:], ones_d)
            load_super(0, 3)

            # ---- per-batch persistent (reused sequentially) ----
            qkT = bigs.tile([128, tt, 4, 128], BF16)    # t-major; rows q1,q2,k1,k2
            v_sb = bigs.tile([128, tt, 260], F32R)      # [tok, v(256)|1|0 pad]
            g_sb = bigs.tile([128, tt, 256], F32)       # gate (raw -> silu'd JIT)

            for b in range(nb):
                # ================= Phase 1 =================
                with tc.tile_pool(name="p1t", bufs=3) as p1t, \
                     tc.tile_pool(name="mm_ps", bufs=3, space="PSUM") as mm_ps:
                    # ones column for every tile in one strided write
                    nc.vector.tensor_copy(v_sb[:, :, 256:260],
                                          _bcast_mid(ones_sb[:], tt))

                    def p1_transp(t, qrot):
                        # q/k transposes entirely on the DMA xbar: no PE
                        # time, no PSUM bank, no copyback
                        for h in range(4):
                            nc.sync.dma_start_transpose(qkT[:, t, h, :],
                                                        qrot[:, h, :])

                    pending_vg = None

                    def p1_vg(t, vg_ps):
                        # v / raw gate copies + SiLU gate; deferred one tile
                        # so the next tile's squares lead the ACT queue
                        nc.scalar.copy(v_sb[:, t, 0:256], vg_ps[:, 0:256])
                        nc.scalar.copy(g_sb[:, t, :], vg_ps[:, 256:512])
                        ge = p1t.tile([128, 256], F32, name="ge")
                        nc.scalar.activation(ge[:], g_sb[:, t, :], AF.Exp,
                                             scale=-1.0)
                        gd = p1t.tile([128, 256], F32, name="gd")
                        nc.gpsimd.tensor_scalar(out=gd[:], in0=ge[:], scalar1=1.0,
                                                scalar2=None, op0=ALU.add)
                        gr = p1t.tile([128, 256], F32, name="gr")
                        nc.vector.reciprocal_approx_fast(out=gr[:], in_=gd[:])
                        nc.gpsimd.tensor_mul(g_sb[:, t, :], g_sb[:, t, :], gr[:])

                    for t in range(tt):
                        s, half = t // 2, t % 2
                        if half == 0 and (b, s) not in xT_pre:
                            load_super(b, s)
                        xT_t = xT_pre[(b, s)]
                        if half == 1:
                            del xT_pre[(b, s)]
                            # prefetch 3 supers ahead
                            if s + 3 < tt // 2 and (b, s + 3) not in xT_pre:
                                load_super(b, s + 3)
                        xsl = slice(half * 128, half * 128 + 128)
                        qk_ps = mm_ps.tile([128, 512], F32, name="qk_ps")
                        vg_ps = mm_ps.tile([128, 512], F32, name="vg_ps")
                        for k in range(KT):
                            nc.tensor.matmul(qk_ps[:], xT_t[:, k, xsl], wcat[:, k, 0:512],
                                             start=(k == 0), stop=(k == KT - 1))
                        for k in range(KT):
                            nc.tensor.matmul(vg_ps[:], xT_t[:, k, xsl], wcat[:, k, 512:1024],
                                             start=(k == 0), stop=(k == KT - 1))
                        # ---- q/k rmsnorm stats FIRST on ACT (they gate the
                        # rsqrt -> qrot -> transpose chain) ----
                        ss = p1t.tile([128, 4], F32, name="ss")
                        sq_scr = p1t.tile([128, 128], F32, name="sq_scr")
                        for h in range(4):
                            nc.scalar.activation(sq_scr[:], qk_ps[:, h * 128:(h + 1) * 128],
                                                 AF.Square, accum_out=ss[:, h:h + 1])
                        # ---- stage qk to SBUF in f32 (frees the PSUM bank
                        # early; single bf16 rounding happens at qrot) ----
                        qksb = p1t.tile([128, 4, 128], F32, name="qksb")
                        nc.scalar.copy(qksb[:],
                                       qk_ps[:].rearrange("p (h d) -> p h d", h=4))
                        h1, h2 = qksb[:, :, 0:64], qksb[:, :, 64:128]
                        cos_b = _bcast_mid(cos_sb[:, t, :], 4)
                        sin_b = _bcast_mid(sin_sb[:, t, :], 4)
                        ra = p1t.tile([128, 4, 64], F32, name="ra")
                        rb = p1t.tile([128, 4, 64], F32, name="rb")
                        rot = p1t.tile([128, 4, 128], F32, name="rot")
                        nc.vector.tensor_mul(ra[:], h1, cos_b)
                        nc.vector.tensor_mul(rb[:], h2, sin_b)
                        nc.vector.tensor_add(rot[:, :, 0:64], ra[:], rb[:])
                        nc.vector.tensor_mul(ra[:], h2, cos_b)
                        nc.vector.tensor_mul(rb[:], h1, sin_b)
                        nc.vector.tensor_sub(rot[:, :, 64:128], ra[:], rb[:])
                        rstd = _rsqrt_dve(nc, p1t, ss[:], 4, HD, "rq", iters=2)
                        qrot = p1t.tile([128, 4, 128], BF16, name="qrot")
                        for h in range(4):
                            nc.vector.tensor_scalar_mul(qrot[:, h, :], in0=rot[:, h, :],
                                                        scalar1=rstd[:, h:h + 1])
                        p1_transp(t, qrot)
                        if pending_vg is not None:
                            p1_vg(*pending_vg)
                        pending_vg = (t, vg_ps)
                    p1_vg(*pending_vg)
                # prefetch next batch's first supers during phase 2
                if b + 1 < nb:
                    for s in range(2):
                        load_super(b + 1, s)
                if phases < 2:
                    with tc.tile_pool(name="dump", bufs=2) as dump:
                        for t in range(tt):
                            d_t = dump.tile([128, 256], F32, name="d_t")
                            nc.vector.tensor_copy(d_t[:], v_sb[:, t, 0:256])
                            nc.vector.tensor_add(d_t[:], d_t[:], g_sb[:, t, :])
                            nc.sync.dma_start(
                                y_d[b, t * 128:(t + 1) * 128, :], d_t[:])
                    continue
                # ================= Phase 2 =================
                with tc.tile_pool(name="sc_ps", bufs=2, space="PSUM") as sc_ps, \
                     tc.tile_pool(name="av_ps", bufs=4, space="PSUM") as av_ps:
                    # --- job list: groups of <=2 score pairs; diagonal is its
                    # own group (needs the causal mask) -------------------
                    groups = []
                    for c in range(nch):
                        per_var = []
                        for var in range(2):
                            gs = []
                            prs_all = list(range(c + 1))
                            for i in range(0, len(prs_all), 2):
                                grp = prs_all[i:i + 2]
                                gs.append((c, var, grp, c in grp))
                            per_var.append(gs)
                        # interleave var streams; keep var0's diag before
                        # var1's diag so the pre-epilogue still leads
                        n = len(per_var[0])
                        for i in range(n):
                            groups.append(per_var[0][i])
                            groups.append(per_var[1][i])

                    sc_tiles = {}

                    def emit_sc(gi):
                        c, var, prs, diag = groups[gi]
                        scp = sc_ps.tile([128, 4, CH], F32, name="sc")
                        qch = qkT[:, 2 * c:2 * c + 2, var, :]
                        for pi, jp in enumerate(prs):
                            for jj in range(2):
                                nc.tensor.matmul(
                                    scp[:, 2 * pi + jj, :],
                                    qkT[:, 2 * jp + jj, 2 + var, :],
                                    qch, start=True, stop=True)
                        sc_tiles[gi] = scp

                    emit_sc(0)
                    yps = {}
                    for gi, (c, var, prs, diag) in enumerate(groups):
                        if var == 0 and prs[0] == 0:
                            for v2 in range(2):
                                for m in range(2):
                                    yps[(v2, m)] = av_ps.tile([128, 260], F32,
                                                              name="yacc")
                        n = 2 * len(prs)
                        scp = sc_tiles.pop(gi)
                        probs = p2s.tile([128, 4, CH], F32R, name="probs")
                        nc.scalar.activation(probs[:, 0:n, :], scp[:, 0:n, :],
                                             AF.Exp, scale=SCALE)
                        if diag:
                            pi = prs.index(c)
                            nc.vector.tensor_mul(
                                probs[:, 2 * pi:2 * pi + 2, :],
                                probs[:, 2 * pi:2 * pi + 2, :], mask_sb[:])
                        # emit next group's scores ahead of this group's AV
                        if gi + 1 < len(groups):
                            emit_sc(gi + 1)
                        for pi, jp in enumerate(prs):
                            for jj in range(2):
                                j = 2 * jp + jj
                                for m in range(2):
                                    if j == 2 * c + 1 and m == 0:
                                        # fully-masked diagonal block: probs
                                        # are exactly zero there -> skip
                                        continue
                                    nc.tensor.matmul(
                                        yps[(var, m)][:],
                                        probs[:, 2 * pi + jj, m * 128:(m + 1) * 128],
                                        v_sb[:, j, :],
                                        start=(j == 0),
                                        stop=(j == 2 * c + 1 - (1 - m)))
                        if diag and var == 0:
                            # var0 accumulators are complete: start the
                            # normalize of y1 while var1's attention runs
                            pre_ep = {}
                            for m in range(2):
                                y1p = yps[(0, m)]
                                r1 = p2e.tile([128, 1], F32, name="r1")
                                nc.vector.reciprocal(r1[:], y1p[:, 256:257])
                                t1 = p2e.tile([128, 256], F32, name="t1")
                                nc.vector.tensor_scalar_mul(
                                    t1[:], in0=y1p[:, 0:256], scalar1=r1[:])
                                pre_ep[m] = t1
                        if not (diag and var == 1):
                            continue
                        # ---- epilogue for chunk c ----
                        ssy = p2e.tile([128, 2], F32, name="ssy")
                        ygs = []
                        for m in range(2):
                            y2p = yps[(1, m)]
                            # v col 257 = -1/lam -> r2n is one recip away
                            r2n = p2e.tile([128, 1], F32, name="r2n")
                            nc.vector.reciprocal(r2n[:], y2p[:, 257:258])
                            t1 = pre_ep[m]
                            yt = p2e.tile([128, 256], F32, name="yt")
                            nc.vector.scalar_tensor_tensor(
                                yt[:], y2p[:, 0:256], r2n[:], t1[:],
                                op0=ALU.mult, op1=ALU.add)
                            yg = p2e.tile([128, 256], F32, name="yg", bufs=2)
                            nc.vector.tensor_mul(yg[:], yt[:],
                                                 g_sb[:, 2 * c + m, :])
                            if c == nch - 1:
                                # tail chunk: ACT is idle; keep DVE chain short
                                sq = p2e.tile([128, 256], F32, name="sq2")
                                nc.scalar.activation(sq[:], yg[:], AF.Square,
                                                     accum_out=ssy[:, m:m + 1])
                            else:
                                sq = p2e.tile([128, 256], F32, name="sq2")
                                nc.vector.tensor_mul(sq[:], yg[:], yg[:])
                                nc.vector.tensor_reduce(
                                    ssy[:, m:m + 1], sq[:],
                                    axis=mybir.AxisListType.X, op=ALU.add)
                            ygs.append(yg)
                        # rsy absorbs the (1-lambda_init) factor:
                        # (ms/C^2)^-0.5 = C * ms^-0.5
                        CI2 = 1.0 / (ONE_MINUS_LI * ONE_MINUS_LI)
                        rsy = _rsqrt_dve(nc, p2e, ssy[:], 2, 256 / CI2, "ry",
                                         iters=2, eps=EPS * CI2)
                        for m in range(2):
                            qt = 2 * c + m
                            out_t = p2e.tile([128, 256], F32, name="out_t")
                            nc.vector.tensor_scalar_mul(
                                out_t[:], in0=ygs[m][:],
                                scalar1=rsy[:, m:m + 1])
                            nc.sync.dma_start(
                                y_d[b, qt * 128:(qt + 1) * 128, :], out_t[:])
    nc.compile()
    return nc


_NC = None


def prep_in_maps(hidden_states, W_qkv, lambda_q1, lambda_k1, lambda_q2,
                 lambda_k2, W_g):
    import ml_dtypes
    bf16 = ml_dtypes.bfloat16
    x = np.asarray(hidden_states, dtype=np.float32)
    xt = np.ascontiguousarray(x.transpose(0, 2, 1)).astype(bf16)
    W_qkv = np.asarray(W_qkv, dtype=np.float32)
    W_g = np.asarray(W_g, dtype=np.float32)

    t_ar = np.arange(T, dtype=np.float32)
    inv_freq = (1.0 / 10000.0 ** (np.arange(0, HD, 2, dtype=np.float32) / HD)
                ).astype(np.float32)
    freqs = np.outer(t_ar, inv_freq).astype(np.float32)
    cos = np.cos(freqs).astype(np.float32)
    sin = np.sin(freqs).astype(np.float32)

    # multiplicative 0/1 causal mask (applied to probs AFTER exp)
    masks = np.empty((128, 2, CH), dtype=np.float32)
    kk = np.arange(128)[:, None]
    qq = np.arange(CH)[None, :]
    for m in range(2):
        masks[:, m, :] = np.where(m * 128 + kk <= qq, 1.0, 0.0)
    
    ident = np.eye(128, dtype=bf16)

    lam1 = np.exp(np.sum(np.asarray(lambda_q1, np.float32)
                         * np.asarray(lambda_k1, np.float32), axis=-1))
    lam2 = np.exp(np.sum(np.asarray(lambda_q2, np.float32)
                         * np.asarray(lambda_k2, np.float32), axis=-1))
    lam = (lam1 - lam2 + LAMBDA_INIT).astype(np.float32)   # [8]

    in_maps = []
    for c in range(N_CORES):
        base = 2 * c * 384
        w_cols = [
            W_qkv[:, base:base + 128],            # q1
            W_qkv[:, base + 384:base + 512],      # q2
            W_qkv[:, base + 128:base + 256],      # k1
            W_qkv[:, base + 512:base + 640],      # k2
            W_qkv[:, base + 256:base + 384],      # v1
            W_qkv[:, base + 640:base + 768],      # v2
            W_g[:, c * 256:(c + 1) * 256],        # gate
        ]
        wcat = np.ascontiguousarray(np.concatenate(w_cols, axis=1)).astype(bf16)
        ones = np.zeros((128, 4), dtype=np.float32)
        ones[:, 0] = 1.0
        ones[:, 1] = -1.0 / lam[c]
        in_maps.append({
            "xt": xt, "wcat": wcat, "cos": cos, "sin": sin,
            "masks": masks, "ident": ident, "ones": ones,
        })

    return in_maps


def kernel(hidden_states, W_qkv, lambda_q1, lambda_k1, lambda_q2, lambda_k2,
           W_g, **run_kwargs):
    global _NC
    if _NC is None:
        _NC = build()
    in_maps = prep_in_maps(hidden_states, W_qkv, lambda_q1, lambda_k1,
                           lambda_q2, lambda_k2, W_g)
    res = run_bass_kernel_spmd(_NC, in_maps, core_ids=list(range(N_CORES)),
                               **run_kwargs)
    out = np.empty((B, T, D), dtype=np.float32)
    for c in range(N_CORES):
        out[:, :, c * 256:(c + 1) * 256] = res.results[c]["y"]
    if run_kwargs:
        return out, res
    return out
